# revision 8
# baseline (speedup 1.0000x reference)
"""TSSA causal self-attention on 8 Trainium2 NeuronCores.

Sharding: 4-way data-parallel over B x 2-way tensor-parallel over heads.
Core c handles batch b = c//2 and heads [8*(c%2), 8*(c%2)+8).

Per core, the kernel runs three phases:
  1) w^T = Wa_half @ x_b^T as (feat, T) tiles (fp32r matmuls), spilling w to
     DRAM, and chains w_sq -> running-denom scan -> reciprocal -> per-head
     column sums (PE ones-reduce) into tmp rows.
  2) One pairwise AllGather of the (8, T) tmp rows, softmax across all 16
     heads (mean-shifted, exp on ACT), extraction of this core's 8 rows via a
     per-core selection matmul, cumsum of Pi and -Pi*cumPi' precomputation.
  3) Re-reads w, rebuilds w_sq, forms den2 = cumPi' + cum(w_sq*Pi) with a
     single fused scan of Pi*(w_sq+1), computes y = w * (-Pi*cumPi') / den2,
     and projects through Wp_half into a partial (T, C) output.
Host sums the two partial outputs of each batch pair.
"""
import sys

sys.path.insert(0, "/opt/trn_rl_repo")

import numpy as np

import concourse.bacc as bacc
import concourse.mybir as mybir
import concourse.tile as tile
from concourse.bass_utils import run_bass_kernel_spmd

B, T, C, H, D = 4, 4096, 2048, 16, 128
NCORES = 8
HPC = H // 2          # heads per core
F = HPC * D           # features per core (1024)
S1 = 512              # stage-1 slab width along T
S2 = 1024             # stage-2 slab width along T
F32 = mybir.dt.float32
FR = mybir.dt.float32r
ADD = mybir.AluOpType.add
MULT = mybir.AluOpType.mult
SUB = mybir.AluOpType.subtract
MAX = mybir.AluOpType.max
BYP = mybir.AluOpType.bypass
GROUPS = [[0, 1], [2, 3], [4, 5], [6, 7]]

_cached_nc = None


def _build():
    nc = bacc.Bacc("TRN2", target_bir_lowering=False, debug=False,
                   num_devices=NCORES)

    xT_d = nc.dram_tensor("xT", [C, T], FR, kind="ExternalInput").ap()
    wa_d = nc.dram_tensor("wa", [C, F], FR, kind="ExternalInput").ap()
    wp_d = nc.dram_tensor("wp", [F, C], FR, kind="ExternalInput").ap()
    bt_d = nc.dram_tensor("btall", [H, T], F32, kind="ExternalInput").ap()
    tl_d = nc.dram_tensor("tmplall", [H, 1], F32, kind="ExternalInput").ap()
    sel_d = nc.dram_tensor("sel", [H, HPC], FR, kind="ExternalInput").ap()
    selx_d = nc.dram_tensor("selx", [HPC, HPC * D], FR,
                            kind="ExternalInput").ap()
    ones_d = nc.dram_tensor("ones", [D, 1], FR, kind="ExternalInput").ap()
    o16m_d = nc.dram_tensor("o16m", [H, H], FR, kind="ExternalInput").ap()
    o16s_d = nc.dram_tensor("o16s", [H, H], FR, kind="ExternalInput").ap()
    out_d = nc.dram_tensor("out_p", [T, C], F32, kind="ExternalOutput").ap()

    with tile.TileContext(nc) as tc:
        with tc.tile_pool(name="dram", bufs=1, space="DRAM") as dp, \
             tc.tile_pool(name="const", bufs=1) as cp:
            w_spill = dp.tile([F, T], F32)
            cc_in = dp.tile([HPC, T], F32)
            cc_out = dp.tile([H, T], F32)

            selx_sb = cp.sbuf_tile_from(selx_d)
            myp = cp.tile([HPC, T], FR)
            npc = cp.tile([HPC, T], FR)

            # ---------------- stage 1 ----------------
            with tc.tile_pool(name="s1w", bufs=1) as pw1, \
                 tc.tile_pool(name="s1x", bufs=20) as px, \
                 tc.tile_pool(name="s1a", bufs=2) as pa, \
                 tc.tile_pool(name="s1ps", bufs=3, space="PSUM") as ps1, \
                 tc.tile_pool(name="s1pc", bufs=2, space="PSUM") as psc:
                wa_sb = pw1.tile([D, 16 * F], FR)
                for k in range(16):
                    nc.sync.dma_start(wa_sb[:, k * F:(k + 1) * F],
                                      wa_d[k * D:(k + 1) * D, :])
                ones = pw1.sbuf_tile_from(ones_d)
                eps12 = pw1.tile([D, 1], F32)
                nc.vector.memset(eps12, 1e-12)
                carry_d = pw1.tile([D, HPC], F32)
                nc.vector.memset(carry_d, 0.0)
                for s in range(T // S1):
                    c0, c1 = s * S1, (s + 1) * S1
                    xs = []
                    for k in range(16):
                        xk = px.tile([D, S1], FR, tag="xs", name=f"xs{s}_{k}")
                        nc.sync.dma_start(xk, xT_d[k * D:(k + 1) * D, c0:c1])
                        xs.append(xk)
                    for h in range(HPC):
                        pw = ps1.tile([D, S1], F32, tag="pw", name=f"pw{s}_{h}")
                        for k in range(16):
                            nc.tensor.matmul(
                                pw,
                                wa_sb[:, k * F + h * D:k * F + (h + 1) * D]
                                ,
                                xs[k],
                                start=(k == 0), stop=(k == 15))
                        w_sb = pa.tile([D, S1], F32, tag="w_sb",
                                       name=f"wsb{s}_{h}")
                        nc.scalar.copy(w_sb, pw)
                        nc.sync.dma_start(
                            w_spill[h * D:(h + 1) * D, c0:c1], w_sb)
                        wsq = pa.tile([D, S1], F32, tag="wsq",
                                      name=f"wsq{s}_{h}")
                        nc.scalar.square(wsq, pw)
                        den = pa.tile([D, S1], F32, tag="den",
                                      name=f"den{s}_{h}")
                        nc.vector.tensor_tensor_scan(
                            den, wsq, eps12.broadcast_to((D, S1)),
                            carry_d[:, h:h + 1], ADD, MAX)
                        nc.vector.tensor_copy(carry_d[:, h:h + 1],
                                              den[:, S1 - 1:S1])
                        rden = pa.tile([D, S1], F32, tag="rden",
                                       name=f"rden{s}_{h}")
                        nc.vector.reciprocal_approx_fast(rden, den)
                        r = pa.tile([D, S1], FR, tag="r", name=f"r{s}_{h}")
                        nc.vector.tensor_tensor(r, wsq, rden, MULT)
                        pc = psc.tile([1, S1], F32, tag="pc",
                                      name=f"pc{s}_{h}")
                        nc.tensor.matmul(pc, ones, r,
                                         start=True, stop=True)
                        trow = pa.tile([1, S1], F32, tag="trow",
                                       name=f"trow{s}_{h}")
                        nc.scalar.copy(trow, pc)
                        nc.sync.dma_start(cc_in[h:h + 1, c0:c1], trow)

            # ---------------- softmax over heads ----------------
            with tc.tile_pool(name="sm", bufs=1) as sm, \
                 tc.tile_pool(name="sm2", bufs=2) as sm2, \
                 tc.tile_pool(name="smps", bufs=2, space="PSUM") as smp:
                ones16m = sm.sbuf_tile_from(o16m_d)
                ones16s = sm.sbuf_tile_from(o16s_d)
                bt_sb = sm.sbuf_tile_from(bt_d)
                tl_sb = sm.sbuf_tile_from(tl_d)
                sel_sb = sm.sbuf_tile_from(sel_d)
                nc.gpsimd.collective_compute(
                    "AllGather", BYP, replica_groups=GROUPS,
                    ins=[cc_in[:, :]], outs=[cc_out[:, :]])
                gat = sm.tile([H, T], F32)
                nc.sync.dma_start(gat, cc_out)
                tadj = sm.tile([H, T], FR)
                nc.vector.scalar_tensor_tensor(tadj, gat, tl_sb, bt_sb,
                                               MULT, ADD)
                esub = sm.tile([H, T], F32)
                for nb in range(T // 512):
                    n0, n1 = nb * 512, (nb + 1) * 512
                    pm = smp.tile([H, 512], F32, tag="pm", name=f"pm{nb}")
                    nc.tensor.matmul(pm, ones16m,
                                     tadj[:, n0:n1],
                                     start=True, stop=True)
                    nc.vector.tensor_tensor(esub[:, n0:n1], tadj[:, n0:n1],
                                            pm, SUB)
                eexp = sm.tile([H, T], FR)
                nc.scalar.activation(eexp, esub,
                                     mybir.ActivationFunctionType.Exp)
                pi = sm.tile([H, T], FR)
                for nb in range(T // 512):
                    n0, n1 = nb * 512, (nb + 1) * 512
                    ps_ = smp.tile([H, 512], F32, tag="ps", name=f"ps{nb}")
                    nc.tensor.matmul(ps_, ones16s,
                                     eexp[:, n0:n1],
                                     start=True, stop=True)
                    rs = sm2.tile([H, 512], F32, tag="rs", name=f"rs{nb}")
                    nc.vector.reciprocal_approx_fast(rs, ps_)
                    nc.vector.tensor_tensor(pi[:, n0:n1], eexp[:, n0:n1],
                                            rs, MULT)
                for nb in range(T // 512):
                    n0, n1 = nb * 512, (nb + 1) * 512
                    mp = smp.tile([HPC, 512], F32, tag="mp", name=f"mp{nb}")
                    nc.tensor.matmul(mp, sel_sb,
                                     pi[:, n0:n1],
                                     start=True, stop=True)
                    nc.scalar.copy(myp[:, n0:n1], mp)
                cpi = sm.tile([HPC, T], F32)
                nc.vector.tensor_tensor_scan(cpi, myp, myp, 1e-8, ADD, BYP)
                nc.vector.scalar_tensor_tensor(npc, myp, -1.0, cpi,
                                               MULT, MULT)

            # ---------------- stage 2 ----------------
            with tc.tile_pool(name="s2w", bufs=1) as pw2, \
                 tc.tile_pool(name="s2a", bufs=2) as p2, \
                 tc.tile_pool(name="s2y", bufs=1) as py, \
                 tc.tile_pool(name="s2bp", bufs=1, space="PSUM") as bps, \
                 tc.tile_pool(name="s2pt", bufs=4, space="PSUM") as pps, \
                 tc.tile_pool(name="s2oc", bufs=3) as poc:
                wp_sb = pw2.tile([D, HPC * C], FR)
                for h in range(HPC):
                    nc.sync.dma_start(wp_sb[:, h * C:(h + 1) * C],
                                      wp_d[h * D:(h + 1) * D, :])
                carry_2 = pw2.tile([D, HPC], F32)
                nc.vector.memset(carry_2, 1e-8)
                for s in range(T // S2):
                    c0 = s * S2
                    ys = []
                    for h in range(HPC):
                        w2 = p2.tile([D, S2], F32, tag="w2", name=f"w2_{s}_{h}")
                        nc.sync.dma_start(
                            w2, w_spill[h * D:(h + 1) * D, c0:c0 + S2])
                        wsq2 = p2.tile([D, S2], F32, tag="wsq2",
                                       name=f"wsq2_{s}_{h}")
                        nc.scalar.square(wsq2, w2)
                        pib = bps.tile([D, S2], F32, tag="pib",
                                       name=f"pib{s}_{h}")
                        npcb = bps.tile([D, S2], F32, tag="npcb",
                                        name=f"npcb{s}_{h}")
                        for nb in range(S2 // 512):
                            n0, n1 = nb * 512, (nb + 1) * 512
                            nc.tensor.matmul(
                                pib[:, n0:n1],
                                selx_sb[:, h * D:(h + 1) * D],
                                myp[:, c0 + n0:c0 + n1],
                                start=True, stop=True)
                            nc.tensor.matmul(
                                npcb[:, n0:n1],
                                selx_sb[:, h * D:(h + 1) * D],
                                npc[:, c0 + n0:c0 + n1],
                                start=True, stop=True)
                        bb = p2.tile([D, S2], F32, tag="bb",
                                     name=f"bb{s}_{h}")
                        nc.vector.scalar_tensor_tensor(bb, wsq2, 1.0, pib,
                                                       ADD, MULT)
                        den2 = p2.tile([D, S2], F32, tag="den2",
                                       name=f"den2_{s}_{h}")
                        nc.vector.tensor_tensor_scan(
                            den2, bb, bb, carry_2[:, h:h + 1], ADD, BYP)
                        nc.vector.tensor_copy(carry_2[:, h:h + 1],
                                              den2[:, S2 - 1:S2])
                        rden2 = p2.tile([D, S2], F32, tag="rden2",
                                        name=f"rden2_{s}_{h}")
                        nc.vector.reciprocal_approx_fast(rden2, den2)
                        num = p2.tile([D, S2], F32, tag="num",
                                      name=f"num{s}_{h}")
                        nc.vector.tensor_tensor(num, w2, npcb, MULT)
                        y_h = py.tile([D, S2], FR, tag=f"y{h}",
                                      name=f"y{s}_{h}")
                        nc.vector.tensor_tensor(y_h, num, rden2, MULT)
                        ys.append(y_h)
                    for tb in range(S2 // D):
                        for ob in range(C // 512):
                            o0, o1 = ob * 512, (ob + 1) * 512
                            pt = pps.tile([D, 512], F32, tag="pt",
                                          name=f"pt{s}_{tb}_{ob}")
                            for h in range(HPC):
                                nc.tensor.matmul(
                                    pt,
                                    ys[h][:, tb * D:(tb + 1) * D],
                                    wp_sb[:, h * C + o0:h * C + o1]
                                    ,
                                    start=(h == 0), stop=(h == HPC - 1))
                            oc = poc.tile([D, 512], F32, tag="oc",
                                          name=f"oc{s}_{tb}_{ob}")
                            nc.scalar.copy(oc, pt)
                            nc.sync.dma_start(
                                out_d[c0 + tb * D:c0 + (tb + 1) * D, o0:o1],
                                oc)

    nc.compile()
    return nc


def _prep_inputs(x, Wa, Wp, temp, denom_bias):
    x = np.asarray(x, dtype=np.float32)
    Wa = np.asarray(Wa, dtype=np.float32)
    Wp = np.asarray(Wp, dtype=np.float32)
    temp = np.asarray(temp, dtype=np.float32)
    denom_bias = np.asarray(denom_bias, dtype=np.float32)

    btall = (D * denom_bias[:, :T, 0] * temp).astype(np.float32)  # (H, T)
    tmplall = temp.reshape(H, 1).astype(np.float32)
    selx = np.zeros((HPC, HPC * D), np.float32)
    for h in range(HPC):
        selx[h, h * D:(h + 1) * D] = 1.0

    in_maps = []
    for c in range(NCORES):
        b, half = c // 2, c % 2
        fsel = slice(half * F, (half + 1) * F)
        sel = np.zeros((H, HPC), np.float32)
        for j in range(HPC):
            sel[half * HPC + j, j] = 1.0
        in_maps.append({
            "xT": np.ascontiguousarray(x[b].T),
            "wa": np.ascontiguousarray(Wa[fsel, :].T),
            "wp": np.ascontiguousarray(Wp[:, fsel].T),
            "btall": btall,
            "tmplall": tmplall,
            "sel": sel,
            "selx": selx,
            "ones": np.ones((D, 1), np.float32),
            "o16m": np.full((H, H), 1.0 / H, np.float32),
            "o16s": np.ones((H, H), np.float32),
        })
    return in_maps


def _run(in_maps, trace=False, tmpdir=None):
    global _cached_nc
    if _cached_nc is None:
        _cached_nc = _build()
    return run_bass_kernel_spmd(_cached_nc, in_maps,
                                core_ids=list(range(NCORES)), trace=trace,
                                tmpdir=tmpdir)


def kernel(x, Wa, Wp, temp, denom_bias):
    in_maps = _prep_inputs(x, Wa, Wp, temp, denom_bias)
    res = _run(in_maps)
    out = np.empty((B, T, C), np.float32)
    for b in range(B):
        out[b] = res.results[2 * b]["out_p"] + res.results[2 * b + 1]["out_p"]
    return out
